# revision 1
# baseline (speedup 1.0000x reference)
"""Trainium2 Bass kernel: 4-layer GPT (B=8,T=512,D=1024,H=16/KV4,FF=4096,V=32000) + LM head.

Sharding: data-parallel over batch — 8 sequences onto 8 NeuronCores, no collectives.
Device kernel keeps activations transposed ([D, T]) so every GEMM is
lhsT=W_tile (stationary), rhs=x^T (moving) with zero on-device transposes.
LayerNorm gains are folded into the following weight matrix on the host;
RoPE runs in a de-interleaved basis (host-permuted Wq/Wk columns) so it is a
32-partition block swap + 3 elementwise ops. Causal attention computes
S^T = K^T-stationary x Q with per-chunk shrinking N (skips fully-masked work),
exp without max-subtraction (scores are O(1) for this model), and gets the
softmax denominator for free from a ones-column appended to V.
"""
import os
import sys
import numpy as np

for _p in ("/opt/trn_rl_repo",):
    if _p not in sys.path:
        sys.path.insert(0, _p)

import concourse.bass as bass
import concourse.mybir as mybir
import concourse.tile as tile
import concourse.bacc as bacc
from concourse.bass_utils import run_bass_kernel_spmd

B, T, D, H, KVH, HD, L, V, FF = 8, 512, 1024, 16, 4, 64, 4, 32000, 4096
P = 128
ND = D // P          # 8 d-tiles
NT = T // P          # 4 token chunks
NFQ = (H * HD) // P  # 8 q feature tiles (head pairs)
NFK = (KVH * HD) // P  # 2 kv feature tiles
NFF = FF // P        # 32
VCH = 500
NV = V // VCH        # 64
NBIAS = NFQ + KVH + ND + NFF + ND  # 60 bias columns per layer (k bias duplicated per kv head)
F32 = mybir.dt.float32
BF16 = mybir.dt.bfloat16
AF = mybir.ActivationFunctionType

LAST_RESULTS = None  # BassKernelResults of the most recent run (for test.py)


# ---------------------------------------------------------------- device ---
def build_program():
    nc = bacc.Bacc(None, target_bir_lowering=False)

    xT_d = nc.dram_tensor("xT", [D, T], F32, kind="ExternalInput")
    wq_d = nc.dram_tensor("wq", [L, NFQ, P, ND * P], BF16, kind="ExternalInput")
    wk_d = nc.dram_tensor("wk", [L, KVH, P, ND * P], BF16, kind="ExternalInput")
    wv_d = nc.dram_tensor("wv", [L, 2, P, 4 * 256], BF16, kind="ExternalInput")
    wcp_d = nc.dram_tensor("wcp", [L, ND, P, ND * P], BF16, kind="ExternalInput")
    wfc_d = nc.dram_tensor("wfc", [L, NFF, P, ND * P], BF16, kind="ExternalInput")
    wpj_d = nc.dram_tensor("wpj", [L, ND, 4, P, 8 * P], BF16, kind="ExternalInput")
    lmw_d = nc.dram_tensor("lmw", [NV, P, ND * VCH], BF16, kind="ExternalInput")
    bias_d = nc.dram_tensor("bias", [L, P, NBIAS], F32, kind="ExternalInput")
    cc4_d = nc.dram_tensor("cc4", [P, T], F32, kind="ExternalInput")
    ss4_d = nc.dram_tensor("ss4", [P, T], F32, kind="ExternalInput")
    dmask_d = nc.dram_tensor("dmask", [P, P], F32, kind="ExternalInput")
    out_d = nc.dram_tensor("out", [T, V], F32, kind="ExternalOutput")

    with tile.TileContext(nc) as tc:
        with (
            tc.tile_pool(name="pers", bufs=1) as pers,
            tc.tile_pool(name="wsmall", bufs=3) as wsmall,
            tc.tile_pool(name="tmp", bufs=2) as tmp,
            tc.tile_pool(name="stage", bufs=4) as stage,
            tc.tile_pool(name="expp", bufs=4) as expp,
            tc.tile_pool(name="statsb", bufs=1) as statsb,
            tc.tile_pool(name="statrr", bufs=2) as statrr,
            tc.tile_pool(name="biasp", bufs=2) as biasp,
            tc.tile_pool(name="psmm", bufs=2, space="PSUM") as psmm,
            tc.tile_pool(name="psy", bufs=2, space="PSUM") as psy,
            tc.tile_pool(name="psstat", bufs=2, space="PSUM") as psstat,
            tc.tile_pool(name="psbc", bufs=2, space="PSUM") as psbc,
        ):
            # persistent tiles
            xT = pers.tile([P, ND * T], F32, tag="xT")
            lnT = pers.tile([P, ND * T], BF16, tag="lnT")
            kT = pers.tile([P, KVH * T], BF16, tag="kT")
            vA = pers.tile([P, NT * 4 * 65], BF16, tag="vA")
            yT = pers.tile([P, ND * T], BF16, tag="yT")
            cc4 = pers.tile([P, T], F32, tag="cc4")
            ss4 = pers.tile([P, T], F32, tag="ss4")
            dmask = pers.tile([P, P], F32, tag="dmask")
            onescol = pers.tile([P, 1], BF16, tag="onescol")
            onesrow = pers.tile([1, P], BF16, tag="onesrow")

            nc.sync.dma_start(cc4[:], cc4_d[:])
            nc.sync.dma_start(ss4[:], ss4_d[:])
            nc.sync.dma_start(dmask[:], dmask_d[:])
            nc.vector.memset(onescol[:], 1.0)
            nc.vector.memset(onesrow[:], 1.0)
            for c in range(NT):
                for h in range(KVH):
                    nc.vector.memset(vA[:, 260 * c + 65 * h + 64 : 260 * c + 65 * h + 65], 1.0)
            for j in range(ND):
                nc.sync.dma_start(xT[:, T * j : T * (j + 1)], xT_d[P * j : P * (j + 1), :])

            def ln_pass(src, dst):
                """dst = (src - mean)/std per column (over the partition x 8-tile D dim)."""
                sum_ps = psstat.tile([1, T], F32, tag="st")
                sq_ps = psstat.tile([1, T], F32, tag="st")
                for j in range(ND):
                    s = src[:, T * j : T * (j + 1)]
                    xb = tmp.tile([P, T], BF16, tag="lnxb")
                    sq = tmp.tile([P, T], BF16, tag="lnsq")
                    nc.vector.tensor_copy(xb[:], s)
                    nc.vector.tensor_mul(sq[:], s, s)
                    nc.tensor.matmul(sum_ps[:], onescol[:], xb[:], start=(j == 0), stop=(j == ND - 1))
                    nc.tensor.matmul(sq_ps[:], onescol[:], sq[:], start=(j == 0), stop=(j == ND - 1))
                m = statsb.tile([1, T], F32, tag="m")
                e2 = statsb.tile([1, T], F32, tag="e2")
                msq = statsb.tile([1, T], F32, tag="msq")
                sd = statsb.tile([1, T], F32, tag="sd")
                arow = statsb.tile([1, T], F32, tag="ar")
                crow = statsb.tile([1, T], F32, tag="cr")
                nc.vector.tensor_scalar_mul(m[:], sum_ps[:], 1.0 / D)
                nc.vector.tensor_scalar_mul(e2[:], sq_ps[:], 1.0 / D)
                nc.vector.tensor_mul(msq[:], m[:], m[:])
                nc.vector.tensor_sub(e2[:], e2[:], msq[:])
                nc.vector.tensor_scalar_add(e2[:], e2[:], 1e-5)
                nc.scalar.activation(sd[:], e2[:], AF.Sqrt)
                nc.vector.reciprocal(arow[:], sd[:])
                nc.vector.tensor_mul(crow[:], m[:], arow[:])
                nc.vector.tensor_scalar_mul(crow[:], crow[:], -1.0)
                arow_b = statsb.tile([1, T], BF16, tag="arb")
                crow_b = statsb.tile([1, T], BF16, tag="crb")
                nc.vector.tensor_copy(arow_b[:], arow[:])
                nc.vector.tensor_copy(crow_b[:], crow[:])
                A_ps = psbc.tile([P, T], F32, tag="bc")
                C_ps = psbc.tile([P, T], F32, tag="bc")
                nc.tensor.matmul(A_ps[:], onesrow[:], arow_b[:], start=True, stop=True)
                nc.tensor.matmul(C_ps[:], onesrow[:], crow_b[:], start=True, stop=True)
                for j in range(ND):
                    d = dst[:, T * j : T * (j + 1)]
                    nc.vector.tensor_mul(d, src[:, T * j : T * (j + 1)], A_ps[:])
                    nc.vector.tensor_add(d, d, C_ps[:])

            def rope(src, dst):
                """dst = src*cc4 + swap32(src)*ss4 (src is clobbered)."""
                sw = tmp.tile([P, T], F32, tag="swp")
                for a, b in ((0, 32), (64, 96)):
                    nc.vector.tensor_copy(sw[a : a + 32, :], src[b : b + 32, :])
                    nc.vector.tensor_copy(sw[b : b + 32, :], src[a : a + 32, :])
                nc.vector.tensor_mul(sw[:], sw[:], ss4[:])
                nc.vector.tensor_mul(src[:], src[:], cc4[:])
                nc.vector.tensor_add(dst, src[:], sw[:])

            def gemm8(w, rhs_tile, ps):
                """ps [P, T] = sum_j w[:, Pj:P(j+1)].T @ rhs_tile[:, Tj:T(j+1)]"""
                for j in range(ND):
                    nc.tensor.matmul(
                        ps[:], w[:, P * j : P * (j + 1)], rhs_tile[:, T * j : T * (j + 1)],
                        start=(j == 0), stop=(j == ND - 1),
                    )

            for l in range(L):
                btile = biasp.tile([P, NBIAS], F32, tag="bias")
                nc.sync.dma_start(btile[:], bias_d[l])
                ln_pass(xT, lnT)

                # --- K: one row-duplicated tile per kv head -> kT (roped) ---
                for i in range(KVH):
                    w = wsmall.tile([P, ND * P], BF16, tag="w")
                    nc.sync.dma_start(w[:], wk_d[l, i])
                    ps = psmm.tile([P, T], F32, tag="mm")
                    gemm8(w, lnT, ps)
                    kraw = tmp.tile([P, T], F32, tag="qraw")
                    nc.scalar.activation(kraw[:], ps[:], AF.Identity,
                                         bias=btile[:, NFQ + i : NFQ + i + 1])
                    rope(kraw, kT[:, T * i : T * (i + 1)])

                # --- V: natural layout [tok, vfeat] + ones column ---
                wv0 = wsmall.tile([P, 4 * 256], BF16, tag="w")
                wv1 = wsmall.tile([P, 4 * 256], BF16, tag="w")
                nc.sync.dma_start(wv0[:], wv_d[l, 0])
                nc.sync.dma_start(wv1[:], wv_d[l, 1])
                for c in range(NT):
                    ps = psmm.tile([P, T], F32, tag="mm")
                    for j in range(ND):
                        wvt = wv0 if j < 4 else wv1
                        nc.tensor.matmul(
                            ps[:, 0:256],
                            lnT[:, T * j + P * c : T * j + P * (c + 1)],
                            wvt[:, 256 * (j % 4) : 256 * (j % 4 + 1)],
                            start=(j == 0), stop=(j == ND - 1),
                        )
                    for h in range(KVH):
                        nc.vector.tensor_copy(vA[:, 260 * c + 65 * h : 260 * c + 65 * h + 64],
                                              ps[:, 64 * h : 64 * h + 64])

                # --- Q + attention, one head-pair tile at a time ---
                for i in range(NFQ):
                    w = wsmall.tile([P, ND * P], BF16, tag="w")
                    nc.sync.dma_start(w[:], wq_d[l, i])
                    ps = psmm.tile([P, T], F32, tag="mm")
                    gemm8(w, lnT, ps)
                    qraw = tmp.tile([P, T], F32, tag="qraw")
                    nc.scalar.activation(qraw[:], ps[:], AF.Identity,
                                         bias=btile[:, i : i + 1])
                    rq = tmp.tile([P, T], BF16, tag="rq")
                    rope(qraw, rq[:])
                    for sub in range(2):
                        h = 2 * i + sub
                        kv = h // 4
                        y_ps = psy.tile([P, T], F32, tag="y")
                        for c in range(NT):
                            N = T - P * c
                            s_ps = psmm.tile([P, T], F32, tag="mm")
                            nc.tensor.matmul(
                                s_ps[:, 0:N],
                                kT[64 * sub : 64 * sub + 64,
                                   T * kv + P * c : T * kv + P * (c + 1)],
                                rq[64 * sub : 64 * sub + 64, P * c : T],
                                start=True, stop=True,
                            )
                            nc.vector.tensor_add(s_ps[:, 0:P], s_ps[:, 0:P], dmask[:])
                            ex = expp.tile([P, T], BF16, tag="ex")
                            nc.scalar.activation(ex[:, 0:N], s_ps[:, 0:N], AF.Exp, scale=0.125)
                            nc.tensor.matmul(
                                y_ps[0:65, P * c : T],
                                vA[:, 260 * c + 65 * kv : 260 * c + 65 * kv + 65],
                                ex[:, 0:N],
                                start=(c == 0), stop=(c == NT - 1),
                            )
                        rrow = statrr.tile([1, T], F32, tag="rr")
                        rrow_b = statrr.tile([1, T], BF16, tag="rrb")
                        nc.vector.reciprocal(rrow[:], y_ps[64:65, :])
                        nc.vector.tensor_copy(rrow_b[:], rrow[:])
                        R_ps = psbc.tile([P, T], F32, tag="bc")
                        nc.tensor.matmul(R_ps[0:64, :], onesrow[0:1, 0:64], rrow_b[:],
                                         start=True, stop=True)
                        R_sb = tmp.tile([P, T], F32, tag="rsb")
                        nc.scalar.copy(R_sb[0:64, :], R_ps[0:64, :])
                        nc.vector.tensor_mul(
                            yT[64 * sub : 64 * sub + 64, T * i : T * (i + 1)],
                            y_ps[0:64, :], R_sb[0:64, :],
                        )

                # --- attention out projection + residual ---
                for fo in range(ND):
                    w = wsmall.tile([P, ND * P], BF16, tag="w")
                    nc.sync.dma_start(w[:], wcp_d[l, fo])
                    ps = psmm.tile([P, T], F32, tag="mm")
                    gemm8(w, yT, ps)
                    ct = tmp.tile([P, T], F32, tag="cptmp")
                    nc.scalar.activation(ct[:], ps[:], AF.Identity,
                                         bias=btile[:, NFQ + KVH + fo : NFQ + KVH + fo + 1])
                    x = xT[:, T * fo : T * (fo + 1)]
                    nc.vector.tensor_add(x, x, ct[:])

                # --- MLP ---
                ln_pass(xT, lnT)
                with tc.tile_pool(name="ffp", bufs=1) as ffp:
                    ffT = ffp.tile([P, NFF * T], BF16, tag="ffT")
                    for f in range(NFF):
                        w = wsmall.tile([P, ND * P], BF16, tag="w")
                        nc.sync.dma_start(w[:], wfc_d[l, f])
                        ps = psmm.tile([P, T], F32, tag="mm")
                        gemm8(w, lnT, ps)
                        bcol = NFQ + KVH + ND + f
                        nc.scalar.activation(ffT[:, T * f : T * (f + 1)], ps[:], AF.Gelu,
                                             bias=btile[:, bcol : bcol + 1])
                    for dout in range(ND):
                        ps = psmm.tile([P, T], F32, tag="mm")
                        for q in range(4):
                            w = wsmall.tile([P, 8 * P], BF16, tag="w")
                            nc.sync.dma_start(w[:], wpj_d[l, dout, q])
                            for f8 in range(8):
                                f = 8 * q + f8
                                nc.tensor.matmul(
                                    ps[:], w[:, P * f8 : P * (f8 + 1)],
                                    ffT[:, T * f : T * (f + 1)],
                                    start=(f == 0), stop=(f == NFF - 1),
                                )
                        bcol = NFQ + KVH + ND + NFF + dout
                        ct = tmp.tile([P, T], F32, tag="cptmp")
                        nc.scalar.activation(ct[:], ps[:], AF.Identity,
                                             bias=btile[:, bcol : bcol + 1])
                        x = xT[:, T * dout : T * (dout + 1)]
                        nc.vector.tensor_add(x, x, ct[:])

            # --- final LN + LM head ---
            ln_pass(xT, lnT)
            with tc.tile_pool(name="wbig", bufs=3) as wbig:
                for v in range(NV):
                    w = wbig.tile([P, ND * VCH], BF16, tag="lw")
                    nc.sync.dma_start(w[:], lmw_d[v])
                    for t in range(NT):
                        ps = psmm.tile([P, T], F32, tag="mm")
                        for j in range(ND):
                            nc.tensor.matmul(
                                ps[:, 0:VCH],
                                lnT[:, T * j + P * t : T * j + P * (t + 1)],
                                w[:, VCH * j : VCH * (j + 1)],
                                start=(j == 0), stop=(j == ND - 1),
                            )
                        st = stage.tile([P, VCH], F32, tag="lmst")
                        nc.vector.tensor_copy(st[:], ps[:, 0:VCH])
                        nc.sync.dma_start(
                            out_d[P * t : P * (t + 1), VCH * v : VCH * (v + 1)], st[:]
                        )

    nc.finalize()
    return nc


# ------------------------------------------------------------------ host ---
def _prep(inputs):
    perm = np.concatenate([np.arange(0, HD, 2), np.arange(1, HD, 2)])
    inv = 1.0 / (10000.0 ** (np.arange(0, HD, 2, dtype=np.float64) / HD))
    ang = inv[:, None] * np.arange(T, dtype=np.float64)[None, :]
    cos_t = np.cos(ang).astype(np.float32)
    sin_t = np.sin(ang).astype(np.float32)
    cc4 = np.ascontiguousarray(np.tile(cos_t, (4, 1)))
    ss4 = np.ascontiguousarray(np.concatenate([-sin_t, sin_t, -sin_t, sin_t], 0))
    r = np.arange(P)
    dmask = np.where(r[:, None] <= r[None, :], 0.0, -1e30).astype(np.float32)

    import ml_dtypes
    f32 = lambda a: np.ascontiguousarray(a, dtype=np.float32)
    bf = lambda a: np.ascontiguousarray(np.asarray(a, dtype=ml_dtypes.bfloat16))
    wq = np.empty((L, NFQ, P, ND * P), np.float32)
    wk = np.empty((L, KVH, P, ND * P), np.float32)
    wv = np.empty((L, 2, P, 4 * 256), np.float32)
    wcp = np.empty((L, ND, P, ND * P), np.float32)
    wfc = np.empty((L, NFF, P, ND * P), np.float32)
    wpj = np.empty((L, ND, 4, P, 8 * P), np.float32)
    bias = np.empty((L, P, NBIAS), np.float32)
    for l in range(L):
        Wa = inputs["ln1_g"][l][:, None] * inputs["c_attn_w"][l]
        ba = inputs["ln1_b"][l] @ inputs["c_attn_w"][l] + inputs["c_attn_b"][l]
        Wq = Wa[:, : H * HD].reshape(D, H, HD)[:, :, perm].reshape(D, H * HD)
        bq = ba[: H * HD].reshape(H, HD)[:, perm].reshape(H * HD)
        Wk = Wa[:, H * HD : H * HD + KVH * HD].reshape(D, KVH, HD)[:, :, perm].reshape(D, KVH * HD)
        bk = ba[H * HD : H * HD + KVH * HD].reshape(KVH, HD)[:, perm].reshape(KVH * HD)
        Wv = Wa[:, H * HD + KVH * HD :]
        bv = ba[H * HD + KVH * HD :]
        bv_exp = np.repeat(bv.reshape(KVH, HD), H // KVH, axis=0).reshape(H * HD)
        bcp = inputs["c_proj_b"][l] + bv_exp @ inputs["c_proj_w"][l]
        Wfc = inputs["ln2_g"][l][:, None] * inputs["fc_w"][l]
        bfc = inputs["ln2_b"][l] @ inputs["fc_w"][l] + inputs["fc_b"][l]
        Wpj, bpj = inputs["proj_w"][l], inputs["proj_b"][l]

        wq[l] = Wq.reshape(ND, P, NFQ, P).transpose(2, 1, 0, 3).reshape(NFQ, P, ND * P)
        # K: one [D, 128] block per kv head with the head's 64 columns duplicated
        # into both output-row halves, so the roped K tile is row-duplicated.
        Wk_dup = np.concatenate(
            [np.tile(Wk[:, HD * kv : HD * (kv + 1)], (1, 2)) for kv in range(KVH)], axis=1
        )  # [D, KVH*128]
        wk[l] = Wk_dup.reshape(ND, P, KVH, P).transpose(2, 1, 0, 3).reshape(KVH, P, ND * P)
        wvr = Wv.reshape(ND, P, 256)
        wv[l, 0] = wvr[0:4].transpose(1, 0, 2).reshape(P, 4 * 256)
        wv[l, 1] = wvr[4:8].transpose(1, 0, 2).reshape(P, 4 * 256)
        wcp[l] = inputs["c_proj_w"][l].reshape(ND, P, ND, P).transpose(2, 1, 0, 3).reshape(ND, P, ND * P)
        wfc[l] = Wfc.reshape(ND, P, NFF, P).transpose(2, 1, 0, 3).reshape(NFF, P, ND * P)
        wpj[l] = Wpj.reshape(4, 8, P, ND, P).transpose(3, 0, 2, 1, 4).reshape(ND, 4, P, 8 * P)
        bk_dup = np.concatenate([np.tile(bk[HD * kv : HD * (kv + 1)], 2) for kv in range(KVH)])
        bias[l] = np.concatenate(
            [bq.reshape(NFQ, P).T, bk_dup.reshape(KVH, P).T, bcp.reshape(ND, P).T,
             bfc.reshape(NFF, P).T, bpj.reshape(ND, P).T], axis=1)

    lmW = inputs["lnf_g"][:, None] * inputs["lm_w"]
    lmw = lmW.reshape(ND, P, NV, VCH).transpose(2, 1, 0, 3).reshape(NV, P, ND * VCH)
    logits_b = inputs["lnf_b"] @ inputs["lm_w"]

    common = dict(
        wq=bf(wq), wk=bf(wk), wv=bf(wv), wcp=bf(wcp), wfc=bf(wfc),
        wpj=bf(wpj), lmw=bf(lmw), bias=f32(bias), cc4=cc4, ss4=ss4, dmask=dmask,
    )
    return common, logits_b


def kernel(**inputs):
    global LAST_RESULTS
    inputs = {k: np.asarray(v) for k, v in inputs.items()}
    ids = inputs["input_ids"].astype(np.int64)
    common, logits_b = _prep(inputs)

    in_maps = []
    for b in range(B):
        xT = np.ascontiguousarray(inputs["wte"][ids[b]].T.astype(np.float32))
        in_maps.append({**common, "xT": xT})

    nc = build_program()
    trace = os.environ.get("KBENCH_TRACE", "0") == "1"
    res = run_bass_kernel_spmd(nc, in_maps, core_ids=list(range(B)), trace=trace)
    LAST_RESULTS = res

    out = np.stack([res.results[b]["out"] for b in range(B)], axis=0)
    if np.any(logits_b != 0.0):
        out = out + logits_b[None, None, :].astype(np.float32)
    return out


if __name__ == "__main__":
    import reference
    inp = {k: np.asarray(v) for k, v in reference.setup_inputs().items()}
    got = kernel(**inp)
    exp = np.asarray(reference.reference(**reference.setup_inputs()))
    rel = np.linalg.norm(got - exp) / np.linalg.norm(exp)
    print("Relative error:", rel)



# revision 10
# speedup vs baseline: 1.2779x; 1.2779x over previous
"""Trainium2 Bass kernel: 4-layer GPT (B=8,T=512,D=1024,H=16/KV4,FF=4096,V=32000) + LM head.

Sharding: data-parallel over batch — 8 sequences onto 8 NeuronCores, no collectives.
Activations stay transposed ([D, T]); weights are bf16, GEMMs W-stationary.

Key structure (v2):
- LayerNorm is *deferred*: Q/K/V GEMMs run on the raw bf16 residual (xb); the
  per-token mean is removed by one rank-1 matmul (colsum(W) x (-mean) row) and
  the per-token 1/std scale A is folded into the rope cos/sin tables (Q/K), a
  per-partition TensorScalar multiply (V, LM head), or applied only where a
  nonlinearity needs it (LN2 -> fc path materializes lnT).
- xb (bf16 residual) and its squares (GpSimd) are produced inside the residual
  loops, so LN stats matmuls can fire immediately at each phase boundary.
- K is computed without row duplication; Q heads are re-paired (host-permuted
  weights) so each kv head lands at the partition offset its paired q needs.
- Q GEMM runs one head-pair ahead of the attention chain (software pipeline).
- Causal mask is a bf16 0/1 multiply on the exp() output, on GpSimd.
- 1/sqrt(var) computed as Exp(-0.5*Ln(var)) to stay in one Act table set.
- Per-phase scoped PSUM pools; weight DMAs coalesced to ~8KB/partition chunks;
  LM logits stream out as bf16.
"""
import os
import sys
import numpy as np

for _p in ("/opt/trn_rl_repo",):
    if _p not in sys.path:
        sys.path.insert(0, _p)

import concourse.bass as bass
import concourse.mybir as mybir
import concourse.tile as tile
import concourse.bacc as bacc
from concourse.bass_utils import run_bass_kernel_spmd

B, T, D, H, KVH, HD, L, V, FF = 8, 512, 1024, 16, 4, 64, 4, 32000, 4096
P = 128
ND = D // P          # 8 d-tiles
NT = T // P          # 4 token chunks
NFQ = (H * HD) // P  # 8 q feature tiles (head pairs)
NFF = FF // P        # 32
VCH = 500
NV = V // VCH        # 64
NBIAS = NFQ + 2 + ND + NFF + ND  # 58 bias columns per layer
NWBAR = NFQ * P + 2 * P + 2 * P  # q tiles, k tiles, v (256)
F32 = mybir.dt.float32
BF16 = mybir.dt.bfloat16
AF = mybir.ActivationFunctionType
PAIR = [0, 4, 1, 5, 2, 6, 3, 7, 8, 12, 9, 13, 10, 14, 11, 15]

LAST_RESULTS = None  # BassKernelResults of the most recent run (for test.py)


# ---------------------------------------------------------------- device ---
def build_program():
    nc = bacc.Bacc(None, target_bir_lowering=False)

    xT_d = nc.dram_tensor("xT", [D, T], F32, kind="ExternalInput")
    wq_d = nc.dram_tensor("wq", [L, 2, P, 4 * ND * P], BF16, kind="ExternalInput")
    wk_d = nc.dram_tensor("wk", [L, P, 2 * ND * P], BF16, kind="ExternalInput")
    wv_d = nc.dram_tensor("wv", [L, P, ND * 256], BF16, kind="ExternalInput")
    wcp_d = nc.dram_tensor("wcp", [L, 2, P, 4 * ND * P], BF16, kind="ExternalInput")
    wfc_d = nc.dram_tensor("wfc", [L, 8, P, 4 * ND * P], BF16, kind="ExternalInput")
    wpj_d = nc.dram_tensor("wpj", [L, ND, P, NFF * P], BF16, kind="ExternalInput")
    lmw_d = nc.dram_tensor("lmw", [NV, P, ND * VCH], BF16, kind="ExternalInput")
    bias_d = nc.dram_tensor("bias", [L, P, NBIAS], F32, kind="ExternalInput")
    wbar_d = nc.dram_tensor("wbar", [L, 1, NWBAR], BF16, kind="ExternalInput")
    ccb_d = nc.dram_tensor("ccb", [P, T], BF16, kind="ExternalInput")
    ssb_d = nc.dram_tensor("ssb", [P, T], BF16, kind="ExternalInput")
    tri_d = nc.dram_tensor("tri", [P, P], BF16, kind="ExternalInput")
    out_d = nc.dram_tensor("out", [T, V], BF16, kind="ExternalOutput")

    with tile.TileContext(nc) as tc:
        with (
            tc.tile_pool(name="pers", bufs=1) as pers,
            tc.tile_pool(name="ropep", bufs=2) as ropep,
            tc.tile_pool(name="ropet", bufs=3) as ropet,
            tc.tile_pool(name="rowp", bufs=1) as rowp,
            tc.tile_pool(name="tmp", bufs=2) as tmp,
            tc.tile_pool(name="expp", bufs=4) as expp,
            tc.tile_pool(name="statrr", bufs=2) as statrr,
            tc.tile_pool(name="biasp", bufs=2) as biasp,
            tc.tile_pool(name="wp", bufs=4) as wp,
            tc.tile_pool(name="stage", bufs=4) as stage,
            tc.tile_pool(name="psmm", bufs=2, space="PSUM") as psmm,
        ):
            # persistent tiles
            xT = pers.tile([P, ND * T], F32, tag="xT")
            xb = pers.tile([P, ND * T], BF16, tag="xb")
            sqT = pers.tile([P, ND * T], BF16, tag="sqT")
            lnT = pers.tile([P, ND * T], BF16, tag="lnT")
            kT = pers.tile([P, 2 * T], BF16, tag="kT")
            vA = pers.tile([P, NT * 4 * 65], BF16, tag="vA")
            yT = pers.tile([P, ND * T], BF16, tag="yT")
            ccb = pers.tile([P, T], BF16, tag="ccb")
            ssb = pers.tile([P, T], BF16, tag="ssb")
            tri = pers.tile([P, P], BF16, tag="tri")
            onescol_m = pers.tile([P, 1], BF16, tag="ocm")  # -1/D
            onescol_s = pers.tile([P, 1], BF16, tag="ocs")  # +1/D
            onesrow_b = pers.tile([1, P], BF16, tag="orb")
            onesrow_f = pers.tile([1, P], F32, tag="orf")
            acol = pers.tile([P, NT], F32, tag="acol")
            epscol = pers.tile([1, 1], F32, tag="eps")

            nc.sync.dma_start(ccb[:], ccb_d[:])
            nc.sync.dma_start(ssb[:], ssb_d[:])
            nc.sync.dma_start(tri[:], tri_d[:])
            nc.vector.memset(onescol_m[:], -1.0 / D)
            nc.vector.memset(onescol_s[:], 1.0 / D)
            nc.vector.memset(onesrow_b[:], 1.0)
            nc.vector.memset(onesrow_f[:], 1.0)
            nc.vector.memset(epscol[:], 1e-5)
            for c in range(NT):
                for h in range(KVH):
                    nc.vector.memset(vA[:, 260 * c + 65 * h + 64 : 260 * c + 65 * h + 65], 1.0)
            for j in range(ND):
                nc.sync.dma_start(xT[:, T * j : T * (j + 1)], xT_d[P * j : P * (j + 1), :])
                nc.scalar.activation(xb[:, T * j : T * (j + 1)],
                                     xT[:, T * j : T * (j + 1)], AF.Identity)
                nc.gpsimd.tensor_mul(sqT[:, T * j : T * (j + 1)],
                                     xT[:, T * j : T * (j + 1)],
                                     xT[:, T * j : T * (j + 1)])

            def ln_stats(psst):
                """negm = -mean, e2 = E[x^2] rows (PSUM) from xb/sqT."""
                s0 = psst.tile([1, T], F32, tag="s0")
                s1 = psst.tile([1, T], F32, tag="s1")
                for j in range(ND):
                    nc.tensor.matmul(s0[:], onescol_m[:], xb[:, T * j : T * (j + 1)],
                                     start=(j == 0), stop=(j == ND - 1))
                    nc.tensor.matmul(s1[:], onescol_s[:], sqT[:, T * j : T * (j + 1)],
                                     start=(j == 0), stop=(j == ND - 1))
                return s0, s1

            def ln_rows(s0, s1):
                """negm_b (bf16 -mean) and arow/arow_b (1/std) from stat rows."""
                negm_b = rowp.tile([1, T], BF16, tag="negmb")
                msq = rowp.tile([1, T], F32, tag="msq")
                var = rowp.tile([1, T], F32, tag="var")
                lnv = rowp.tile([1, T], F32, tag="lnv")
                arow = rowp.tile([1, T], F32, tag="arow")
                arow_b = rowp.tile([1, T], BF16, tag="arowb")
                nc.vector.tensor_copy(negm_b[:], s0[:])
                nc.vector.tensor_mul(msq[:], negm_b[:], negm_b[:])
                nc.vector.tensor_sub(var[:], s1[:], msq[:])
                # 1/sqrt(var+eps) = exp(-0.5*ln(var+eps)): stays in the
                # natural_log_exp table set (Sqrt would force a table switch)
                nc.scalar.activation(lnv[:], var[:], AF.Ln, bias=epscol[:])
                nc.scalar.activation(arow[:], lnv[:], AF.Exp, scale=-0.5)
                nc.vector.tensor_copy(arow_b[:], arow[:])
                return negm_b, arow, arow_b

            def a_col(arow, psac):
                """acol[:, t] = arow[0, 128t:128(t+1)].T via tiny fp32 matmuls."""
                ac_ps = psac.tile([P, NT], F32, tag="ac")
                for t in range(NT):
                    nc.tensor.matmul(ac_ps[:, t : t + 1],
                                     arow[0:1, P * t : P * (t + 1)],
                                     onesrow_f[0:1, 0:1],
                                     start=(t == 0), stop=True)
                nc.vector.tensor_copy(acol[:], ac_ps[:])

            def gemm8(w, woff, rhs_tile, ps, stop_last=False):
                for j in range(ND):
                    nc.tensor.matmul(
                        ps[:], w[:, woff + P * j : woff + P * (j + 1)],
                        rhs_tile[:, T * j : T * (j + 1)],
                        start=(j == 0), stop=(stop_last and j == ND - 1),
                    )

            def rope(src, ccA, ssA, dst):
                """dst = src*ccA + swap32(src)*ssA (all bf16; src clobbered)."""
                sw = ropet.tile([P, T], BF16, tag="swp")
                for a, b in ((0, 32), (64, 96)):
                    nc.vector.tensor_copy(sw[a : a + 32, :], src[b : b + 32, :])
                    nc.vector.tensor_copy(sw[b : b + 32, :], src[a : a + 32, :])
                nc.vector.tensor_mul(sw[:], sw[:], ssA[:])
                nc.vector.tensor_mul(src[:], src[:], ccA[:])
                nc.vector.tensor_add(dst, src[:], sw[:])

            def emit_xb(j):
                """bf16 copy + squares of the residual tile j (after update)."""
                nc.scalar.activation(xb[:, T * j : T * (j + 1)],
                                     xT[:, T * j : T * (j + 1)], AF.Identity)
                nc.gpsimd.tensor_mul(sqT[:, T * j : T * (j + 1)],
                                     xT[:, T * j : T * (j + 1)],
                                     xT[:, T * j : T * (j + 1)])

            for l in range(L):
                btile = biasp.tile([P, NBIAS], F32, tag="bias")
                wbar = biasp.tile([1, NWBAR], BF16, tag="wbar")
                nc.sync.dma_start(btile[:], bias_d[l])
                nc.sync.dma_start(wbar[:], wbar_d[l])

                # ---- LN1 (deferred): stats + A-folded rope tables + acol ----
                with (
                    tc.tile_pool(name="psst", bufs=1, space="PSUM") as psst,
                    tc.tile_pool(name="psbc", bufs=1, space="PSUM") as psbc,
                    tc.tile_pool(name="psac", bufs=1, space="PSUM") as psac,
                ):
                    s0, s1 = ln_stats(psst)
                    negm_b, arow, arow_b = ln_rows(s0, s1)
                    A_ps = psbc.tile([P, T], F32, tag="bcA")
                    nc.tensor.matmul(A_ps[:], onesrow_b[:], arow_b[:], start=True, stop=True)
                    ccA = ropep.tile([P, T], BF16, tag="ccA")
                    ssA = ropep.tile([P, T], BF16, tag="ssA")
                    nc.vector.tensor_mul(ccA[:], ccb[:], A_ps[:])
                    nc.vector.tensor_mul(ssA[:], ssb[:], A_ps[:])
                    a_col(arow, psac)

                # --- K: two tiles (kv pair each), rank-1 mean fix ---
                wk = wp.tile([P, 2 * ND * P], BF16, tag="w")
                nc.sync.dma_start(wk[:], wk_d[l])
                for p in range(2):
                    ps = psmm.tile([P, T], F32, tag="mm")
                    gemm8(wk, ND * P * p, xb, ps)
                    nc.tensor.matmul(ps[:], wbar[0:1, (NFQ + p) * P : (NFQ + p + 1) * P],
                                     negm_b[:], start=False, stop=True)
                    kraw = ropet.tile([P, T], BF16, tag="qraw")
                    nc.scalar.activation(kraw[:], ps[:], AF.Identity,
                                         bias=btile[:, NFQ + p : NFQ + p + 1])
                    rope(kraw, ccA, ssA, kT[:, T * p : T * (p + 1)])

                # --- V: natural layout [tok, vfeat], A-scaled via acol ---
                wv = wp.tile([P, ND * 256], BF16, tag="w")
                nc.sync.dma_start(wv[:], wv_d[l])
                for c in range(NT):
                    ps = psmm.tile([P, T], F32, tag="mm")
                    for j in range(ND):
                        nc.tensor.matmul(
                            ps[:, 0:256],
                            xb[:, T * j + P * c : T * j + P * (c + 1)],
                            wv[:, 256 * j : 256 * (j + 1)],
                            start=(j == 0), stop=False,
                        )
                    nc.tensor.matmul(ps[:, 0:256],
                                     negm_b[0:1, P * c : P * (c + 1)],
                                     wbar[0:1, (NFQ + 2) * P : (NFQ + 2) * P + 256],
                                     start=False, stop=True)
                    for h in range(KVH):
                        nc.vector.tensor_scalar_mul(
                            vA[:, 260 * c + 65 * h : 260 * c + 65 * h + 64],
                            ps[:, 64 * h : 64 * h + 64], acol[:, c : c + 1])

                # --- Q + attention: q GEMM pipelined one head-pair ahead ---
                with (
                    tc.tile_pool(name="pss", bufs=2, space="PSUM") as pss,
                    tc.tile_pool(name="psy", bufs=2, space="PSUM") as psy,
                    tc.tile_pool(name="psr", bufs=1, space="PSUM") as psr,
                ):
                    def q_tile(i):
                        if i % 4 == 0:
                            q_tile.w = wp.tile([P, 4 * ND * P], BF16, tag="w")
                            nc.sync.dma_start(q_tile.w[:], wq_d[l, i // 4])
                        ps = psmm.tile([P, T], F32, tag="mm")
                        gemm8(q_tile.w, ND * P * (i % 4), xb, ps)
                        nc.tensor.matmul(ps[:], wbar[0:1, i * P : (i + 1) * P],
                                         negm_b[:], start=False, stop=True)
                        qraw = ropet.tile([P, T], BF16, tag="qraw")
                        nc.scalar.activation(qraw[:], ps[:], AF.Identity,
                                             bias=btile[:, i : i + 1])
                        rq = ropet.tile([P, T], BF16, tag="rq")
                        rope(qraw, ccA, ssA, rq[:])
                        return rq

                    rq = q_tile(0)
                    for i in range(NFQ):
                        rq_next = q_tile(i + 1) if i + 1 < NFQ else None
                        for sub in range(2):
                            kv = PAIR[2 * i + sub] // 4
                            y_ps = psy.tile([P, T], F32, tag="y")
                            for c in range(NT):
                                N = T - P * c
                                s_ps = pss.tile([P, T], F32, tag="s")
                                nc.tensor.matmul(
                                    s_ps[:, 0:N],
                                    kT[64 * sub : 64 * sub + 64,
                                       T * (i // 4) + P * c : T * (i // 4) + P * (c + 1)],
                                    rq[64 * sub : 64 * sub + 64, P * c : T],
                                    start=True, stop=True,
                                )
                                ex = expp.tile([P, T], BF16, tag="ex")
                                nc.scalar.activation(ex[:, 0:N], s_ps[:, 0:N], AF.Exp, scale=0.125)
                                nc.gpsimd.tensor_mul(ex[:, 0:P], ex[:, 0:P], tri[:])
                                nc.tensor.matmul(
                                    y_ps[0:65, P * c : T],
                                    vA[:, 260 * c + 65 * kv : 260 * c + 65 * kv + 65],
                                    ex[:, 0:N],
                                    start=(c == 0), stop=(c == NT - 1),
                                )
                            rrow = statrr.tile([1, T], F32, tag="rr")
                            nc.vector.reciprocal(rrow[:], y_ps[64:65, :])
                            rrow_b = statrr.tile([1, T], BF16, tag="rrb")
                            nc.vector.tensor_copy(rrow_b[:], rrow[:])
                            R_ps = psr.tile([P, T], F32, tag="r")
                            nc.tensor.matmul(R_ps[0:64, :], onesrow_b[0:1, 0:64],
                                             rrow_b[:], start=True, stop=True)
                            R_sb = tmp.tile([P, T], F32, tag="rsb")
                            nc.scalar.copy(R_sb[0:64, :], R_ps[0:64, :])
                            nc.vector.tensor_mul(
                                yT[64 * sub : 64 * sub + 64, T * i : T * (i + 1)],
                                y_ps[0:64, :], R_sb[0:64, :],
                            )
                        rq = rq_next

                # --- attention out projection + residual (+ xb/sq refresh) ---
                for fo in range(ND):
                    if fo % 4 == 0:
                        wcp = wp.tile([P, 4 * ND * P], BF16, tag="w")
                        nc.sync.dma_start(wcp[:], wcp_d[l, fo // 4])
                    ps = psmm.tile([P, T], F32, tag="mm")
                    gemm8(wcp, ND * P * (fo % 4), yT, ps, stop_last=True)
                    ct = tmp.tile([P, T], F32, tag="cptmp")
                    nc.scalar.activation(ct[:], ps[:], AF.Identity,
                                         bias=btile[:, NFQ + 2 + fo : NFQ + 2 + fo + 1])
                    x = xT[:, T * fo : T * (fo + 1)]
                    nc.vector.tensor_add(x, x, ct[:])
                    emit_xb(fo)

                # ---- LN2 (materialized lnT for the gelu path) ----
                with (
                    tc.tile_pool(name="psst", bufs=1, space="PSUM") as psst,
                    tc.tile_pool(name="psbc", bufs=1, space="PSUM") as psbc,
                ):
                    s0, s1 = ln_stats(psst)
                    negm_b, arow, arow_b = ln_rows(s0, s1)
                    A_ps = psbc.tile([P, T], F32, tag="bcA")
                    nc.tensor.matmul(A_ps[:], onesrow_b[:], arow_b[:], start=True, stop=True)
                    M_ps = psbc.tile([P, T], F32, tag="bcM")
                    nc.tensor.matmul(M_ps[:], onesrow_b[:], negm_b[:], start=True, stop=True)
                    A_sb = tmp.tile([P, T], BF16, tag="asb")
                    M_sb = tmp.tile([P, T], BF16, tag="msb")
                    nc.scalar.copy(A_sb[:], A_ps[:])
                    nc.scalar.copy(M_sb[:], M_ps[:])
                    for j in range(ND):
                        t = tmp.tile([P, T], BF16, tag="lnx")
                        nc.vector.tensor_add(t[:], xb[:, T * j : T * (j + 1)], M_sb[:])
                        nc.vector.tensor_mul(lnT[:, T * j : T * (j + 1)], t[:], A_sb[:])

                # --- MLP (proj loop refreshes xb/sq for the next LN) ---
                with tc.tile_pool(name="ffp", bufs=1) as ffp:
                    ffT = ffp.tile([P, NFF * T], BF16, tag="ffT")
                    for fcc in range(8):
                        wfc = wp.tile([P, 4 * ND * P], BF16, tag="w")
                        nc.sync.dma_start(wfc[:], wfc_d[l, fcc])
                        for f4 in range(4):
                            f = 4 * fcc + f4
                            ps = psmm.tile([P, T], F32, tag="mm")
                            gemm8(wfc, ND * P * f4, lnT, ps, stop_last=True)
                            bcol = NFQ + 2 + ND + f
                            nc.scalar.activation(ffT[:, T * f : T * (f + 1)], ps[:], AF.Gelu,
                                                 bias=btile[:, bcol : bcol + 1])
                    for dout in range(ND):
                        wpj = wp.tile([P, NFF * P], BF16, tag="w")
                        nc.sync.dma_start(wpj[:], wpj_d[l, dout])
                        ps = psmm.tile([P, T], F32, tag="mm")
                        for f in range(NFF):
                            nc.tensor.matmul(
                                ps[:], wpj[:, P * f : P * (f + 1)],
                                ffT[:, T * f : T * (f + 1)],
                                start=(f == 0), stop=(f == NFF - 1),
                            )
                        bcol = NFQ + 2 + ND + NFF + dout
                        ct = tmp.tile([P, T], F32, tag="cptmp")
                        nc.scalar.activation(ct[:], ps[:], AF.Identity,
                                             bias=btile[:, bcol : bcol + 1])
                        x = xT[:, T * dout : T * (dout + 1)]
                        nc.vector.tensor_add(x, x, ct[:])
                        emit_xb(dout)

            # --- final LN (mean into xb, scale deferred to acol) + LM head ---
            with (
                tc.tile_pool(name="psst", bufs=1, space="PSUM") as psst,
                tc.tile_pool(name="psbc", bufs=1, space="PSUM") as psbc,
                tc.tile_pool(name="psac", bufs=1, space="PSUM") as psac,
            ):
                s0, s1 = ln_stats(psst)
                negm_b, arow, arow_b = ln_rows(s0, s1)
                M_ps = psbc.tile([P, T], F32, tag="bcM")
                nc.tensor.matmul(M_ps[:], onesrow_b[:], negm_b[:], start=True, stop=True)
                M_sb = tmp.tile([P, T], BF16, tag="msb")
                nc.scalar.copy(M_sb[:], M_ps[:])
                a_col(arow, psac)
                for j in range(ND):
                    nc.vector.tensor_add(xb[:, T * j : T * (j + 1)],
                                         xb[:, T * j : T * (j + 1)], M_sb[:])

                for v in range(NV):
                    w = wp.tile([P, ND * VCH], BF16, tag="w")
                    nc.sync.dma_start(w[:], lmw_d[v])
                    for t in range(NT):
                        ps = psmm.tile([P, T], F32, tag="mm")
                        for j in range(ND):
                            nc.tensor.matmul(
                                ps[:, 0:VCH],
                                xb[:, T * j + P * t : T * j + P * (t + 1)],
                                w[:, VCH * j : VCH * (j + 1)],
                                start=(j == 0), stop=(j == ND - 1),
                            )
                        st = stage.tile([P, VCH], BF16, tag="lmst")
                        nc.vector.tensor_scalar_mul(st[:], ps[:, 0:VCH], acol[:, t : t + 1])
                        nc.sync.dma_start(
                            out_d[P * t : P * (t + 1), VCH * v : VCH * (v + 1)], st[:]
                        )

    nc.finalize()
    return nc


# ------------------------------------------------------------------ host ---
def _prep(inputs):
    perm = np.concatenate([np.arange(0, HD, 2), np.arange(1, HD, 2)])
    inv = 1.0 / (10000.0 ** (np.arange(0, HD, 2, dtype=np.float64) / HD))
    ang = inv[:, None] * np.arange(T, dtype=np.float64)[None, :]
    cos_t = np.cos(ang).astype(np.float32)
    sin_t = np.sin(ang).astype(np.float32)
    cc4 = np.ascontiguousarray(np.tile(cos_t, (4, 1)))
    ss4 = np.ascontiguousarray(np.concatenate([-sin_t, sin_t, -sin_t, sin_t], 0))
    r = np.arange(P)
    tri = (r[:, None] <= r[None, :]).astype(np.float32)

    import ml_dtypes
    f32 = lambda a: np.ascontiguousarray(a, dtype=np.float32)
    bf = lambda a: np.ascontiguousarray(np.asarray(a, dtype=ml_dtypes.bfloat16))
    wq = np.empty((L, 2, P, 4 * ND * P), np.float32)
    wk = np.empty((L, P, 2 * ND * P), np.float32)
    wv = np.empty((L, P, ND * 256), np.float32)
    wcp = np.empty((L, 2, P, 4 * ND * P), np.float32)
    wfc = np.empty((L, 8, P, 4 * ND * P), np.float32)
    wpj = np.empty((L, ND, P, NFF * P), np.float32)
    bias = np.empty((L, P, NBIAS), np.float32)
    wbar = np.empty((L, 1, NWBAR), np.float32)
    for l in range(L):
        Wa = inputs["ln1_g"][l][:, None] * inputs["c_attn_w"][l]
        ba = inputs["ln1_b"][l] @ inputs["c_attn_w"][l] + inputs["c_attn_b"][l]
        # Q: rope-permuted within head, heads re-paired per PAIR
        Wq = Wa[:, : H * HD].reshape(D, H, HD)[:, :, perm][:, PAIR].reshape(D, H * HD)
        bq = ba[: H * HD].reshape(H, HD)[:, perm][PAIR].reshape(H * HD)
        Wk = Wa[:, H * HD : H * HD + KVH * HD].reshape(D, KVH, HD)[:, :, perm].reshape(D, KVH * HD)
        bk = ba[H * HD : H * HD + KVH * HD].reshape(KVH, HD)[:, perm].reshape(KVH * HD)
        Wv = Wa[:, H * HD + KVH * HD :]
        bv = ba[H * HD + KVH * HD :]
        bv_exp = np.repeat(bv.reshape(KVH, HD), H // KVH, axis=0).reshape(H * HD)
        bcp = inputs["c_proj_b"][l] + bv_exp @ inputs["c_proj_w"][l]
        Wfc = inputs["ln2_g"][l][:, None] * inputs["fc_w"][l]
        bfc = inputs["ln2_b"][l] @ inputs["fc_w"][l] + inputs["fc_b"][l]
        Wpj, bpj = inputs["proj_w"][l], inputs["proj_b"][l]

        wq_tiles = Wq.reshape(ND, P, NFQ, P).transpose(2, 1, 0, 3).reshape(NFQ, P, ND * P)
        wq[l] = wq_tiles.reshape(2, 4, P, ND * P).transpose(0, 2, 1, 3).reshape(2, P, 4 * ND * P)
        wk_tiles = Wk.reshape(ND, P, 2, P).transpose(2, 1, 0, 3).reshape(2, P, ND * P)
        wk[l] = wk_tiles.transpose(1, 0, 2).reshape(P, 2 * ND * P)
        wv[l] = Wv.reshape(ND, P, 256).transpose(1, 0, 2).reshape(P, ND * 256)
        # c_proj rows regrouped to the paired yT feature order
        Wcp_r = inputs["c_proj_w"][l].reshape(H, HD, D)[PAIR].reshape(D, D)
        wcp_tiles = Wcp_r.reshape(ND, P, ND, P).transpose(2, 1, 0, 3).reshape(ND, P, ND * P)
        wcp[l] = wcp_tiles.reshape(2, 4, P, ND * P).transpose(0, 2, 1, 3).reshape(2, P, 4 * ND * P)
        wfc_tiles = Wfc.reshape(ND, P, NFF, P).transpose(2, 1, 0, 3).reshape(NFF, P, ND * P)
        wfc[l] = wfc_tiles.reshape(8, 4, P, ND * P).transpose(0, 2, 1, 3).reshape(8, P, 4 * ND * P)
        wpj[l] = Wpj.reshape(NFF, P, ND, P).transpose(2, 1, 0, 3).reshape(ND, P, NFF * P)
        bias[l] = np.concatenate(
            [bq.reshape(NFQ, P).T, bk.reshape(2, P).T, bcp.reshape(ND, P).T,
             bfc.reshape(NFF, P).T, bpj.reshape(ND, P).T], axis=1)
        wbar[l, 0] = np.concatenate([Wq.sum(0), Wk.sum(0), Wv.sum(0)])

    lmW = inputs["lnf_g"][:, None] * inputs["lm_w"]
    lmw = lmW.reshape(ND, P, NV, VCH).transpose(2, 1, 0, 3).reshape(NV, P, ND * VCH)
    logits_b = inputs["lnf_b"] @ inputs["lm_w"]

    common = dict(
        wq=bf(wq), wk=bf(wk), wv=bf(wv), wcp=bf(wcp), wfc=bf(wfc),
        wpj=bf(wpj), lmw=bf(lmw), bias=f32(bias), wbar=bf(wbar),
        ccb=bf(cc4), ssb=bf(ss4), tri=bf(tri),
    )
    return common, logits_b


def kernel(**inputs):
    global LAST_RESULTS
    inputs = {k: np.asarray(v) for k, v in inputs.items()}
    ids = inputs["input_ids"].astype(np.int64)
    common, logits_b = _prep(inputs)

    in_maps = []
    for b in range(B):
        xT = np.ascontiguousarray(inputs["wte"][ids[b]].T.astype(np.float32))
        in_maps.append({**common, "xT": xT})

    nc = build_program()
    trace = os.environ.get("KBENCH_TRACE", "0") == "1"
    res = run_bass_kernel_spmd(nc, in_maps, core_ids=list(range(B)), trace=trace)
    LAST_RESULTS = res

    out = np.stack([np.asarray(res.results[b]["out"], dtype=np.float32) for b in range(B)], axis=0)
    if np.any(logits_b != 0.0):
        out = out + logits_b[None, None, :].astype(np.float32)
    return out


if __name__ == "__main__":
    import reference
    inp = {k: np.asarray(v) for k, v in reference.setup_inputs().items()}
    got = kernel(**inp)
    exp = np.asarray(reference.reference(**reference.setup_inputs()))
    rel = np.linalg.norm(got - exp) / np.linalg.norm(exp)
    print("Relative error:", rel)
